# revision 1
# baseline (speedup 1.0000x reference)
"""Distributed Trainium2 kernel for nn_CompareLoss (8 NeuronCores).

Math (validated against the reference):
  z = [strong; weak]  (2B x D), s = z / ||z||  (row-normalized)
  logits(i,j) = (s_i . s_j) / tau,  pos_i = logits(i, B+i) = logits(B+i, i)
  Every row r contributes  ln(e^{pos_r} + sum_{j in C(r)} e^{logits(r,j)})
  - pos_r, with column set C(r):
    - positive rows (strong_i / weak_{B+i}, i < P): C = all 2N negative rows
    - negative rows (i >= P):                       C = the P strong-pos rows
  loss = (sum over all 2B rows) / (2B).  Logits bounded by 1/tau -> no
  max-subtraction needed in the LSE.

Sharding: data-parallel over the pair index. Core c owns 256 positive and
256 negative pairs -> 1024 rows/core.  Each core gets the full column set
feature-major (z^T, fp16) with its own row blocks rotated to the front so
one SPMD program serves all 8 cores.  No collectives; the host sums 8
tiny partial tensors.

Device pipeline (engine-balanced around the ACT exp floor of ~24us):
  - zt loads stream in 8 chunked DMAs; the M2-critical prefix
    [sn|wn|spos|wp] (2816 cols) first.
  - Column norms: DVE squares (one 3D [128,2,w] op per chunk) -> paired
    ones-matmuls whose lhsT VALUE is 1/tau (runtime, broadcast via a K=1
    matmul), accumulating invtau*ssq into a multi-partition [rows,512]
    psum tile -> DVE reciprocal -> one tiny ACT Sqrt -> rn rows, written
    flat to DRAM once and partition-broadcast back in 3+4 wide DMAs.
    This keeps ACT's non-exp work ~1.5us total (the baseline spent ~17us
    in single-partition Ln/Exp passes + 5 table swaps).
  - 12 main jobs [128,2048]: 8 fp16 matmuls + one ACT Exp with fused
    row-sum (accum_out).  The exp outputs are written to one write-only
    scratch tile; only the row sums (ACC[128,12]) leave the device.
  - Raw pos-pair logits ([1,512] psum) ship to the host as well; the
    host does the final ln(S + e^pos) - pos reduction in float64 (cheap:
    12K values/core), removing the tail Ln + its table load.
  - PE is kept continuously busy from ~6.5us (junk warm-up matmuls) so
    the tensor engine reaches its max p-state before the real matmuls.
  - DMA triggers cost ~600ns each on the issuing queue: zt + broadcast +
    output triggers all live on the Sync queue (idle otherwise); the
    Scalar queue runs pure ACT so exps are never blocked behind DMA.
  - PSUM is exactly 8 banks: every psum tile shares one 2-slot pool
    ([128,2048] slots); iv/ssqA/ssqB rotate through before the mains.
"""

import numpy as np

B = 4096
D = 256
P = 2048
NCORES = 8
IC = P // NCORES          # 256 pair-indices per core (per pos/neg half)
NCOL = 3 * P + IC         # 6400 columns

# column layout: [sn 256 | wn 256 | spos 2048 | wp 256 | snr 1792 | wnr 1792]
OFF_SN = 0
OFF_WN = 256
OFF_SPOS = 512
OFF_WP = 2560
OFF_SNR = 2816
OFF_WNR = 4608

# zt DMA chunks (start, width): M2-critical prefix first, then G2
ZT_CHUNKS = [(0, 512), (512, 1024), (1536, 1024), (2560, 256),
             (2816, 1024), (3840, 1024), (4864, 1024), (5888, 512)]
N_G1 = 4                  # first 4 chunks = G1 (cols 0:2816)

# ssq rows: 512 columns per psum partition row
G1_COLS = 2816            # ssA rows 0..5 (row5 cols 0:256 used)
G2_COLS = 3584            # ssB rows 0..6

# rn broadcast ranges (dst start, width, flat tensor, flat offset)
BCA = [(0, 1024, 0), (1024, 1024, 1024), (2048, 768, 2048)]
BCB = [(2816, 1024, 0), (3840, 1024, 1024), (4864, 1024, 2048),
       (5888, 512, 3072)]

# main jobs: (lhsT col, [4 rhs 512-col slice starts], ACC col)
M2_JOBS = [(OFF_SN, [512, 1024, 1536, 2048], 0),
           (OFF_SN + 128, [512, 1024, 1536, 2048], 1),
           (OFF_WN, [512, 1024, 1536, 2048], 2),
           (OFF_WN + 128, [512, 1024, 1536, 2048], 3)]
# the 8 negative 512-col slices: [0:512] = sn+wn, the rest = snr+wnr.
# Each M1 lhsT tile covers slice-set A in its first job and B in its
# second (order within a job is irrelevant - only the row-sum is kept).
NEG_A = [0, 2816, 3328, 3840]
NEG_B = [4352, 4864, 5376, 5888]
M1_JOBS = [(OFF_SPOS, NEG_A, 4),
           (OFF_SPOS + 128, NEG_A, 6),
           (OFF_WP, NEG_A, 8),
           (OFF_WP + 128, NEG_A, 10)]
M1_JOBS_2 = [(OFF_SPOS, NEG_B, 5),
             (OFF_SPOS + 128, NEG_B, 7),
             (OFF_WP, NEG_B, 9),
             (OFF_WP + 128, NEG_B, 11)]

_CACHE: dict = {}


def _build_nc():
    import concourse.bacc as bacc
    import concourse.tile as tile
    from concourse import mybir
    from concourse.tile_rust import add_dep_helper

    f32 = mybir.dt.float32
    f16 = mybir.dt.float16
    EXP = mybir.ActivationFunctionType.Exp
    SQRT = mybir.ActivationFunctionType.Sqrt

    nc = bacc.Bacc("TRN2", target_bir_lowering=False, debug=False,
                   num_devices=NCORES)
    zt_d = nc.dram_tensor("zt", [2 * D // 2, NCOL], f16, kind="ExternalInput")
    iv_d = nc.dram_tensor("tauv", [1, 1], f32, kind="ExternalInput")
    acc_d = nc.dram_tensor("acc", [128, 12], f32, kind="ExternalOutput")
    pos_d = nc.dram_tensor("pos", [1, 512], f32, kind="ExternalOutput")

    zt3 = zt_d[:, :].rearrange("(h p) c -> p h c", h=2)  # [128,2,NCOL] view

    with tile.TileContext(nc) as tc:
        with (
            tc.tile_pool(name="const", bufs=1) as constp,
            tc.tile_pool(name="big", bufs=1) as bigp,
            tc.tile_pool(name="work", bufs=3) as workp,
            tc.tile_pool(name="dram", bufs=1, space="DRAM") as dramp,
            tc.tile_pool(name="ps", bufs=2, space="PSUM") as psp,
        ):
            # ---------------- input DMAs (sync queue) ----------------
            zt = bigp.tile([128, 2, NCOL], f16)
            ivt = constp.tile([1, 1], f32)
            d_iv = nc.sync.dma_start(ivt[:], iv_d[:])
            prev = [d_iv]
            for s, w in ZT_CHUNKS:
                d = nc.sync.dma_start(zt[:, :, s:s + w], zt3[:, :, s:s + w])
                # chain with depth 2 so chunks arrive in order at full
                # bandwidth instead of round-robining to a joint finish
                if len(prev) >= 2:
                    add_dep_helper(d.ins, prev[-2].ins, sync=True,
                                   reason="zt chunk ordering")
                prev.append(d)

            # ---------------- constants ----------------
            ones16_1 = constp.tile([1, 128], f16)
            nc.gpsimd.memset(ones16_1[:], 1.0)
            ones16_k = constp.tile([128, 1], f16)
            nc.gpsimd.memset(ones16_k[:], 1.0)
            junkW = constp.tile([128, 128], f16)
            nc.gpsimd.memset(junkW[:], 0.0)
            junkR = constp.tile([128, 1], f16)
            nc.gpsimd.memset(junkR[:], 0.0)

            # tau broadcast to [128,1] via K=1 matmul.  EZ is the
            # sumsq lhsT: sliding 13-col windows of [0*13 | tau | 0*12]
            # place tau*ssq of chunk r at psum partition r (recip then
            # gives invtau/ssq, sqrt of that is the rn scale; matmul
            # output must start at partition 0, so the row index comes
            # from the hot column's position inside the window).
            iv16 = constp.tile([1, 1], f16)
            nc.vector.tensor_copy(iv16[:], ivt[:])
            iv_ps = psp.tile([128, 1], f32, tag="ps")
            nc.tensor.matmul(iv_ps[:], ones16_1[0:1, :], iv16[0:1, 0:1],
                             start=True, stop=True)
            EZ = constp.tile([128, 26], f16)
            nc.gpsimd.memset(EZ[:], 0.0)
            nc.vector.tensor_copy(EZ[:, 13:14], iv_ps[:])

            # PE warm-up: keep the tensor engine continuously busy so it
            # ramps to max p-state before the real matmuls (ldweights of
            # a [128,128] tile dominates; results overwrite iv_ps, dead)
            for _ in range(12):
                nc.tensor.matmul(iv_ps[:, 0:1], junkW[:], junkR[0:128, 0:1],
                                 start=True, stop=True)

            # ---------------- column sumsq -> rn ----------------
            # squares per chunk (one 3D DVE op), then EZ-window matmuls
            # accumulating invtau*ssq of 512-col subchunk r into psum
            # partition r of a single [13,512] tile (1 bank).  The first
            # matmul (start=True, full width) zero-fills all rows, so the
            # reciprocal never reads uninitialized psum.
            ssA = psp.tile([13, 512], f32, tag="ps")
            ssB = psp.tile([13, 512], f32, tag="ps")

            def do_chunk(s, w, grp_base, ss, first, last):
                sq = workp.tile([128, 2, 1024], f16, tag="sq",
                                name=f"sq_{s}")
                nc.vector.tensor_mul(sq[:, :, 0:w], zt[:, :, s:s + w],
                                     zt[:, :, s:s + w])
                lo = 0
                while lo < w:
                    r = (s - grp_base + lo) // 512
                    ww = min(512, w - lo)
                    for h in range(2):
                        nc.tensor.matmul(
                            ss[0:13, 0:ww], EZ[:, 13 - r:26 - r],
                            sq[:, h, lo:lo + ww],
                            start=(first and lo == 0 and h == 0),
                            stop=(last and lo + ww >= w and h == 1))
                    lo += ww

            for i, (s, w) in enumerate(ZT_CHUNKS[:N_G1]):
                do_chunk(s, w, 0, ssA, i == 0, i == N_G1 - 1)

            # G1 rn: DVE reciprocal -> ACT sqrt -> f16 rows
            rsqA = constp.tile([6, 512], f32)
            nc.vector.reciprocal(rsqA[:], ssA[0:6, :])
            rn_tA = constp.tile([6, 512], f16)
            nc.scalar.activation(rn_tA[:], rsqA[:], SQRT)
            flatA = dramp.tile([1, 3072], f16)
            d_fA = nc.sync.dma_start(
                flatA[0:1, :].rearrange("o (p c) -> p (o c)", p=6), rn_tA[:])

            # G2 chunks (squares + matmuls run as data lands)
            ng2 = len(ZT_CHUNKS) - N_G1
            for i, (s, w) in enumerate(ZT_CHUNKS[N_G1:]):
                do_chunk(s, w, OFF_SNR, ssB, i == 0, i == ng2 - 1)

            rnb = bigp.tile([128, NCOL], f16)
            bcA = []
            for ds, w, fo in BCA:
                d = nc.sync.dma_start(rnb[:, ds:ds + w],
                                      flatA[0:1, fo:fo + w]
                                      .to_broadcast((128, w)))
                add_dep_helper(d.ins, d_fA.ins, sync=True,
                               reason="bcast after flat write")
                bcA.append(d)

            # G1 normalize (DVE), lhsT/pos chunks first via range order
            ztn = bigp.tile([128, 2, NCOL], f16)
            for ds, w, _ in BCA:
                for h in range(2):
                    nc.vector.tensor_mul(ztn[:, h, ds:ds + w],
                                         zt[:, h, ds:ds + w],
                                         rnb[:, ds:ds + w])

            # G2 rn
            rsqB = constp.tile([7, 512], f32)
            nc.vector.reciprocal(rsqB[:], ssB[0:7, :])
            rn_tB = constp.tile([7, 512], f16)
            nc.scalar.activation(rn_tB[:], rsqB[:], SQRT)
            flatB = dramp.tile([1, 3584], f16)
            d_fB = nc.sync.dma_start(
                flatB[0:1, :].rearrange("o (p c) -> p (o c)", p=7), rn_tB[:])
            for ds, w, fo in BCB:
                d = nc.sync.dma_start(rnb[:, ds:ds + w],
                                      flatB[0:1, fo:fo + w]
                                      .to_broadcast((128, w)))
                add_dep_helper(d.ins, d_fB.ins, sync=True,
                               reason="bcast after flat write")

            # ---------------- main similarity jobs ----------------
            ACC = constp.tile([128, 12], f32)
            esc = constp.tile([128, 2048], f16)   # write-only exp sink

            def main_job(lhs_off, rhs_list, acccol):
                ps = psp.tile([128, 2048], f32, tag="ps",
                              name=f"mm{acccol}")
                for h4, c0 in enumerate(rhs_list):
                    for h in range(2):
                        nc.tensor.matmul(
                            ps[:, h4 * 512:(h4 + 1) * 512],
                            ztn[:, h, lhs_off:lhs_off + 128],
                            ztn[:, h, c0:c0 + 512],
                            start=(h == 0), stop=(h == 1))
                nc.scalar.activation(esc[:], ps[:], EXP,
                                     accum_out=ACC[:, acccol:acccol + 1])

            for lhs_off, rhs_list, acccol in M2_JOBS:
                main_job(lhs_off, rhs_list, acccol)

            # G2 normalize interleaves with the M2 jobs on the DVE
            for ds, w, _ in BCB:
                for h in range(2):
                    nc.vector.tensor_mul(ztn[:, h, ds:ds + w],
                                         zt[:, h, ds:ds + w],
                                         rnb[:, ds:ds + w])

            for lhs_off, rhs_list, acccol in M1_JOBS:
                main_job(lhs_off, rhs_list, acccol)
            for lhs_off, rhs_list, acccol in M1_JOBS_2:
                main_job(lhs_off, rhs_list, acccol)

            # ---------------- raw pos-pair logits ----------------
            # products of normalized columns; summed over k by ones-matmul
            pr_pos = workp.tile([128, 2, 256], f16, tag="pr")
            nc.vector.tensor_mul(pr_pos[:],
                                 ztn[:, :, OFF_SPOS:OFF_SPOS + 256],
                                 ztn[:, :, OFF_WP:OFF_WP + 256])
            pr_neg = workp.tile([128, 2, 256], f16, tag="pr")
            nc.vector.tensor_mul(pr_neg[:],
                                 ztn[:, :, OFF_SN:OFF_SN + 256],
                                 ztn[:, :, OFF_WN:OFF_WN + 256])
            pos_ps = psp.tile([1, 512], f32, tag="ps")
            for half, pr in ((0, pr_pos), (1, pr_neg)):
                o = half * 256
                nc.tensor.matmul(pos_ps[0:1, o:o + 256], ones16_k[:],
                                 pr[:, 0, :], start=True, stop=False)
                nc.tensor.matmul(pos_ps[0:1, o:o + 256], ones16_k[:],
                                 pr[:, 1, :], start=False, stop=True)
            pos_sb = constp.tile([1, 512], f32)
            nc.vector.tensor_copy(pos_sb[:], pos_ps[:])

            # ---------------- outputs ----------------
            nc.sync.dma_start(acc_d[:], ACC[:])
            nc.sync.dma_start(pos_d[:], pos_sb[:])

    nc.compile()
    return nc


def get_nc():
    if "nc" not in _CACHE:
        _CACHE["nc"] = _build_nc()
    return _CACHE["nc"]


def make_in_maps(strong: np.ndarray, weak: np.ndarray, temp: np.ndarray):
    """Host-side sharding: slice + rotate + transpose (pure data movement)."""
    tauv = np.asarray(temp, np.float32).reshape(1, 1)
    in_maps = []
    for c in range(NCORES):
        r = c * IC
        sneg = np.roll(strong[P:B], -r, axis=0)
        wneg = np.roll(weak[P:B], -r, axis=0)
        spos = np.roll(strong[0:P], -r, axis=0)
        wp = weak[r:r + IC]
        cols = np.concatenate([sneg[0:IC], wneg[0:IC], spos, wp,
                               sneg[IC:], wneg[IC:]], axis=0)
        zt = np.ascontiguousarray(cols.T.astype(np.float16))
        in_maps.append({"zt": zt, "tauv": tauv})
    return in_maps


def kernel(inputs, strong_inputs, targets, num_pos, temperature):
    assert int(num_pos) == P
    strong = np.ascontiguousarray(np.asarray(strong_inputs, dtype=np.float32))
    weak = np.ascontiguousarray(np.asarray(inputs, dtype=np.float32))
    temp = np.asarray(temperature, dtype=np.float32).reshape(1, 1)

    from concourse.bass_utils import run_bass_kernel_spmd

    nc = get_nc()
    in_maps = make_in_maps(strong, weak, temp)
    res = run_bass_kernel_spmd(nc, in_maps, core_ids=list(range(NCORES)))
    return finish_host(res.results)


def finish_host(results):
    """Final ln(S + e^pos) - pos reduction in float64 on the host."""
    total = 0.0
    for r in results:
        acc = np.asarray(r["acc"], np.float64)      # [128, 12]
        pos = np.asarray(r["pos"], np.float64).reshape(512)
        p = np.arange(128)
        # M2 rows: ACC cols 0..3 = sn0, sn1, wn0, wn1
        for c in range(4):
            q = pos[256 + (c % 2) * 128 + p]
            total += np.sum(np.log(acc[:, c] + np.exp(q)) - q)
        # M1 rows: ACC cols 4+2t, 5+2t = the two halves of tile t
        for t in range(4):
            q = pos[(t % 2) * 128 + p]
            s = acc[:, 4 + 2 * t] + acc[:, 5 + 2 * t]
            total += np.sum(np.log(s + np.exp(q)) - q)
    return np.float32(total / (2 * B))



# revision 7
# speedup vs baseline: 1.1612x; 1.1612x over previous
"""Distributed Trainium2 kernel for nn_CompareLoss (8 NeuronCores), v2.

Math (validated against the reference):
  z = [strong; weak] (2B x D), s = z/||z||, logits(i,j) = (s_i.s_j)/tau.
  The whole loss reduces to exps of the [2P x 2N] matrix L with rows
  [spos; wpos] and cols [sneg; wneg]:
    loss1 row sums   = row sums of exp(L)               (all 2P rows)
    loss2 "col" sums = col sums of exp(L[:P, :])        (spos rows only)
  plus the positive-pair logits p_i = s_spos_i.s_wpos_i and
  q_j = s_sneg_j.s_wneg_j.  Host does the final ln(S+e^p)-p reduction in
  float64 (tiny).  This exploits sim symmetry: the baseline recomputed
  the sneg/wneg-row x spos-col blocks (25.2M exps); here 16.8M only.

Sharding: 2D grid, 4 row-groups x 2 col-groups.  Core (r,g) owns 512
pos-pair rows (spos/wpos slab r) and 1024 neg cols (sneg/wneg group g,
rolled by -256r so each core owns a disjoint 256-slice of neg pairs).
Columns per core: [spos 512 | sneg 1024 | wneg 1024 | wpos 512] = 3072.
No collectives; the host sums the tiny partials.

Device pipeline (engine-balanced around the ACT exp floor of ~20us):
  - zt loads in 4 chunked DMAs (sync queue, depth-2 chained).
  - Column sumsq: DVE squares -> tau-valued window matmuls (EZt) into a
    single [6,512] psum tile -> rn = Exp(-0.5*Ln(tau*ssq)) on ACT.  Ln
    and Exp share ONE table set (natural_log_exp_and_others) so there is
    exactly one ACT table load, forced at t=0 by a dummy Ln.
  - rn broadcast WITHOUT DMA: 6 one-hot [6x128] matmuls replicate rn_t
    row j across 128 partitions into psum pieces; DVE normalizes zt
    directly against the psum pieces (no SBUF copy).
  - 16 main tiles [128,1024]: 2 fp16 matmuls each + ACT Exp with fused
    row-sum (accum_out -> ACC[128,16]).  spos-tile exps write real fp16
    values; window matmuls (OZ1) accumulate their col sums into rows
    0..3 of a [6,512] psum tile; pos/neg pair logits land in rows 4..5.
  - Outputs: ACC [128,16] and cs [6,512] f32; host finishes in f64.
  - PSUM budget: small pool 1 bank (warm/iv/ssq/cs rotate) + main pool
    3x2 banks = 7 of 8 banks.
"""

import numpy as np

B = 4096
D = 256
P = 2048
NCORES = 8
RG = 4                    # row groups (pos-pair slabs of 512)
CG = 2                    # col groups (neg slabs of 1024)
SLAB = P // RG            # 512 pos pairs per row-group
CGN = P // CG             # 1024 negs per col-group
NCOL = 2 * SLAB + 2 * CGN  # 3072

OFF_SPOS = 0
OFF_SN = 512
OFF_WN = 1536
OFF_WPOS = 2560

ZT_CHUNKS = [(0, 512), (512, 1024), (1536, 1024), (2560, 512)]
NSSQ = NCOL // 512        # 6 ssq rows of 512 cols

_CACHE: dict = {}


def _build_nc():
    import concourse.bacc as bacc
    import concourse.tile as tile
    from concourse import mybir
    from concourse.tile_rust import add_dep_helper

    f32 = mybir.dt.float32
    f16 = mybir.dt.float16
    EXP = mybir.ActivationFunctionType.Exp
    LN = mybir.ActivationFunctionType.Ln

    nc = bacc.Bacc("TRN2", target_bir_lowering=False, debug=False,
                   num_devices=NCORES)
    zt_d = nc.dram_tensor("zt", [128, 2 * NCOL], f16, kind="ExternalInput")
    iv_d = nc.dram_tensor("tauv", [1, 1], f32, kind="ExternalInput")
    obb_d = nc.dram_tensor("obb", [6, 6 * 128], f16, kind="ExternalInput")
    acc_d = nc.dram_tensor("acc", [128, 16], f32, kind="ExternalOutput")
    cs_d = nc.dram_tensor("cs", [6, 512], f32, kind="ExternalOutput")

    zt3 = zt_d[:, :].rearrange("p (h c) -> p h c", h=2)  # [128,2,NCOL] view

    with tile.TileContext(nc) as tc:
        with (
            tc.tile_pool(name="const", bufs=1) as constp,
            tc.tile_pool(name="big", bufs=1) as bigp,
            tc.tile_pool(name="work", bufs=3) as workp,
            tc.tile_pool(name="esc", bufs=2) as escp,
            tc.tile_pool(name="sps", bufs=1, space="PSUM") as sps,
            tc.tile_pool(name="mps", bufs=3, space="PSUM") as mps,
        ):
            # ---------------- input DMAs (sync queue) ----------------
            zt = bigp.tile([128, 2, NCOL], f16)
            ivt = constp.tile([1, 1], f32)
            dmas = []
            for i, (s, w) in enumerate(ZT_CHUNKS):
                d = nc.sync.dma_start(zt[:, :, s:s + w], zt3[:, :, s:s + w])
                if len(dmas) >= 2:
                    add_dep_helper(d.ins, dmas[-2].ins, sync=True,
                                   reason="zt chunk ordering")
                dmas.append(d)
                if i == 0:
                    dmas.append(nc.sync.dma_start(ivt[:], iv_d[:]))

            # ---------------- constants (gpsimd memsets) ----------------
            ones16_1 = constp.tile([1, 128], f16)
            nc.gpsimd.memset(ones16_1[:], 1.0)
            junkW = constp.tile([128, 128], f16)
            nc.gpsimd.memset(junkW[:], 0.0)
            junkR = constp.tile([128, 512], f16)
            nc.gpsimd.memset(junkR[:], 0.0)
            # EZt: tau at col 6 (windows of width 6 place tau*ssq of
            # 512-chunk r at psum partition r).  OZ1: same but value 1.0
            # for exp col-sums / pair sums.  OBB: one-hot lhsT patterns
            # for the rn partition-broadcast matmuls.
            EZt = constp.tile([128, 12], f16)
            nc.gpsimd.memset(EZt[:], 0.0)
            OZ1 = constp.tile([128, 12], f16)
            nc.gpsimd.memset(OZ1[:], 0.0)
            nc.gpsimd.memset(OZ1[:, 6:7], 1.0)
            OBB = constp.tile([6, 6 * 128], f16)
            nc.sync.dma_start(OBB[:], obb_d[:])
            dum1 = constp.tile([1, 1], f16)
            nc.gpsimd.memset(dum1[:], 1.0)

            # dummy Ln at t=0: forces the single table load
            # (natural_log_exp_and_others covers Ln AND Exp) during DMA.
            dumo = constp.tile([1, 1], f32)
            nc.scalar.activation(dumo[:], dum1[:], LN)

            # ---------------- PE warm-up + tau bootstrap ----------------
            warm = sps.tile([128, 512], f32, tag="sps")
            for _ in range(8):
                nc.tensor.matmul(warm[:], junkW[:], junkR[:],
                                 start=True, stop=True)
            iv16 = constp.tile([1, 1], f16)
            nc.vector.tensor_copy(iv16[:], ivt[:])
            iv_ps = sps.tile([128, 1], f32, tag="sps")
            nc.tensor.matmul(iv_ps[:], ones16_1[0:1, :], iv16[0:1, 0:1],
                             start=True, stop=True)
            nc.vector.tensor_copy(EZt[:, 6:7], iv_ps[:])

            # ---------------- column sumsq ----------------
            sq = bigp.tile([128, 2, NCOL], f16)
            ssq = sps.tile([6, 512], f32, tag="sps")
            for ci, (s, w) in enumerate(ZT_CHUNKS):
                nc.vector.tensor_mul(sq[:, :, s:s + w], zt[:, :, s:s + w],
                                     zt[:, :, s:s + w])
                lo = s
                while lo < s + w:
                    r = lo // 512
                    for h in range(2):
                        nc.tensor.matmul(
                            ssq[0:6, 0:512], EZt[:, 6 - r:12 - r],
                            sq[:, h, lo:lo + 512],
                            start=(r == 0 and h == 0),
                            stop=(r == NSSQ - 1 and h == 1))
                    lo += 512

            # rn = exp(-0.5 * ln(tau * ssq)) = 1/(n*sqrt(tau)), on ACT only
            lnt = constp.tile([6, 512], f32)
            nc.scalar.activation(lnt[:], ssq[0:6, :], LN)
            rn_t = constp.tile([6, 512], f16)
            nc.scalar.activation(rn_t[:], lnt[:], EXP, scale=-0.5)

            # ---------------- rn partition-broadcast via PE ----------------
            # piece k covers cols [ds,ds+w); rnb stays in PSUM (f32) and
            # normalize reads it directly (DVE 1x from PSUM, no copy).
            ztn = bigp.tile([128, 2, NCOL], f16)
            for (ds, w) in ZT_CHUNKS:
                rp = mps.tile([128, 1024], f32, tag="mps", name=f"rnb{ds}")
                for jj in range(w // 512):
                    j = (ds + 512 * jj) // 512
                    nc.tensor.matmul(
                        rp[:, 512 * jj:512 * jj + 512],
                        OBB[:, 128 * j:128 * j + 128], rn_t[0:6, :],
                        start=True, stop=True)
                for h in range(2):
                    nc.vector.tensor_mul(ztn[:, h, ds:ds + w],
                                         zt[:, h, ds:ds + w],
                                         rp[:, 0:w])

            # ---------------- main tiles ----------------
            ACC = constp.tile([128, 16], f32)
            escJ = constp.tile([128, 1024], f16)   # wpos exp sink
            cs = sps.tile([6, 512], f32, tag="sps")
            cs_started = [False]

            def cs_mm(row, rhs_ap, stop=False):
                nc.tensor.matmul(cs[0:6, 0:rhs_ap.shape[-1]],
                                 OZ1[:, 6 - row:12 - row], rhs_ap,
                                 start=not cs_started[0], stop=stop,
                                 skip_group_check=True)
                cs_started[0] = True

            def main_tile(T, lhs_off, rhs_off, is_spos, u):
                ps = mps.tile([128, 1024], f32, tag="mps", name=f"mm{T}")
                for c2 in range(2):
                    for h in range(2):
                        nc.tensor.matmul(
                            ps[:, 512 * c2:512 * c2 + 512],
                            ztn[:, h, lhs_off:lhs_off + 128],
                            ztn[:, h, rhs_off + 512 * c2:
                                rhs_off + 512 * c2 + 512],
                            start=(h == 0), stop=(h == 1))
                if is_spos:
                    esc = escp.tile([128, 1024], f16, tag="esc",
                                    name=f"esc{T}")
                    nc.scalar.activation(esc[:], ps[:], EXP,
                                         accum_out=ACC[:, T:T + 1])
                    for c in range(2):
                        cs_mm(2 * u + c, esc[:, 512 * c:512 * c + 512])
                else:
                    nc.scalar.activation(escJ[:], ps[:], EXP,
                                         accum_out=ACC[:, T:T + 1])

            for t in range(4):
                for u in range(2):
                    main_tile(2 * t + u, OFF_SPOS + 128 * t,
                              OFF_SN + 1024 * u, True, u)

            # pair logits (DVE products + window matmuls into cs rows 4/5)
            pr_pos = workp.tile([128, 2, 512], f16, tag="pr")
            nc.vector.tensor_mul(pr_pos[:],
                                 ztn[:, :, OFF_SPOS:OFF_SPOS + 512],
                                 ztn[:, :, OFF_WPOS:OFF_WPOS + 512])
            pr_neg = workp.tile([128, 2, 256], f16, tag="pr")
            nc.vector.tensor_mul(pr_neg[:],
                                 ztn[:, :, OFF_SN:OFF_SN + 256],
                                 ztn[:, :, OFF_WN:OFF_WN + 256])

            for t in range(4):
                for u in range(2):
                    main_tile(8 + 2 * t + u, OFF_WPOS + 128 * t,
                              OFF_SN + 1024 * u, False, u)
                if t == 0:
                    # squeeze pair matmuls into the wpos-phase PE slack
                    cs_mm(4, pr_pos[:, 0, :])
                    cs_mm(4, pr_pos[:, 1, :])
                    cs_mm(5, pr_neg[:, 0, :])
                    cs_mm(5, pr_neg[:, 1, :], stop=True)

            # ---------------- outputs ----------------
            csb = constp.tile([6, 512], f32)
            nc.vector.tensor_copy(csb[:], cs[:])
            nc.sync.dma_start(acc_d[:], ACC[:])
            nc.sync.dma_start(cs_d[:], csb[:])

    nc.compile()
    return nc


def get_nc():
    if "nc" not in _CACHE:
        _CACHE["nc"] = _build_nc()
    return _CACHE["nc"]


def make_in_maps(strong: np.ndarray, weak: np.ndarray, temp: np.ndarray):
    """Host-side sharding: slice + roll + transpose (pure data movement)."""
    tauv = np.asarray(temp, np.float32).reshape(1, 1)
    obb = np.zeros((6, 6 * 128), np.float16)
    for j in range(NSSQ):
        obb[j, 128 * j:128 * j + 128] = 1.0
    in_maps = []
    for r in range(RG):
        for g in range(CG):
            spos = strong[SLAB * r:SLAB * r + SLAB]
            wpos = weak[SLAB * r:SLAB * r + SLAB]
            sneg = np.roll(strong[P + CGN * g:P + CGN * g + CGN],
                           -256 * r, axis=0)
            wneg = np.roll(weak[P + CGN * g:P + CGN * g + CGN],
                           -256 * r, axis=0)
            cols = np.concatenate([spos, sneg, wneg, wpos], axis=0)
            zt16 = cols.T.astype(np.float16)              # [256, 3072]
            ztd = np.ascontiguousarray(
                zt16.reshape(2, 128, NCOL).transpose(1, 0, 2)
                .reshape(128, 2 * NCOL))
            in_maps.append({"zt": ztd, "tauv": tauv, "obb": obb})
    return in_maps


def kernel(inputs, strong_inputs, targets, num_pos, temperature):
    assert int(num_pos) == P
    strong = np.ascontiguousarray(np.asarray(strong_inputs, dtype=np.float32))
    weak = np.ascontiguousarray(np.asarray(inputs, dtype=np.float32))
    temp = np.asarray(temperature, dtype=np.float32).reshape(1, 1)

    from concourse.bass_utils import run_bass_kernel_spmd

    nc = get_nc()
    in_maps = make_in_maps(strong, weak, temp)
    res = run_bass_kernel_spmd(nc, in_maps, core_ids=list(range(NCORES)))
    return finish_host(res.results)


def finish_host(results):
    """Final ln(S + e^p) - p reduction in float64 on the host."""
    S1s = np.zeros((RG, SLAB))
    S1w = np.zeros((RG, SLAB))
    CA = np.zeros((CG, CGN))
    CB = np.zeros((CG, CGN))
    pos_l = np.zeros((RG, SLAB))
    neg_l = np.zeros((CG, CGN))
    for r in range(RG):
        for g in range(CG):
            res = results[CG * r + g]
            acc = np.asarray(res["acc"], np.float64)     # [128, 16]
            cs = np.asarray(res["cs"], np.float64)       # [6, 512]
            for t in range(4):
                sl = slice(128 * t, 128 * t + 128)
                S1s[r, sl] += acc[:, 2 * t] + acc[:, 2 * t + 1]
                S1w[r, sl] += acc[:, 8 + 2 * t] + acc[:, 8 + 2 * t + 1]
            CA[g] += np.roll(cs[0:2].reshape(CGN), 256 * r)
            CB[g] += np.roll(cs[2:4].reshape(CGN), 256 * r)
            if g == 0:
                pos_l[r] = cs[4]
            neg_l[g, 256 * r:256 * r + 256] = cs[5, 0:256]
    p = pos_l.reshape(-1)
    q = neg_l.reshape(-1)
    ep, eq = np.exp(p), np.exp(q)
    total = (np.sum(np.log(S1s.reshape(-1) + ep) - p)
             + np.sum(np.log(S1w.reshape(-1) + ep) - p)
             + np.sum(np.log(CA.reshape(-1) + eq) - q)
             + np.sum(np.log(CB.reshape(-1) + eq) - q))
    return np.float32(total / (2 * B))


# revision 18
# speedup vs baseline: 1.2783x; 1.1008x over previous
"""Distributed Trainium2 kernel for nn_CompareLoss (8 NeuronCores), v2.

Math (validated against the reference):
  z = [strong; weak] (2B x D), s = z/||z||, logits(i,j) = (s_i.s_j)/tau.
  The whole loss reduces to exps of the [2P x 2N] matrix L with rows
  [spos; wpos] and cols [sneg; wneg]:
    loss1 row sums   = row sums of exp(L)               (all 2P rows)
    loss2 "col" sums = col sums of exp(L[:P, :])        (spos rows only)
  plus the positive-pair logits p_i = s_spos_i.s_wpos_i and
  q_j = s_sneg_j.s_wneg_j.  Host does the final ln(S+e^p)-p reduction in
  float64 (tiny).  This exploits sim symmetry: the baseline recomputed
  the sneg/wneg-row x spos-col blocks (25.2M exps); here 16.8M only.

Sharding: 2D grid, 4 row-groups x 2 col-groups.  Core (r,g) owns 512
pos-pair rows (spos/wpos slab r) and 1024 neg cols (sneg/wneg group g,
rolled by -256r so each core owns a disjoint 256-slice of neg pairs).
Columns per core: [spos 512 | sneg 1024 | wneg 1024 | wpos 512] = 3072.
No collectives; the host sums the tiny partials.

Device pipeline (engine-balanced around the ACT exp floor of ~20us):
  - zt loads in 4 chunked DMAs (sync queue, depth-2 chained).
  - Column sumsq: DVE squares -> tau-valued window matmuls (EZt) into a
    single [6,512] psum tile -> rn = Exp(-0.5*Ln(tau*ssq)) on ACT.  Ln
    and Exp share ONE table set (natural_log_exp_and_others) so there is
    exactly one ACT table load, forced at t=0 by a dummy Ln.
  - rn broadcast WITHOUT DMA: 6 one-hot [6x128] matmuls replicate rn_t
    row j across 128 partitions into psum pieces; DVE normalizes zt
    directly against the psum pieces (no SBUF copy).
  - 16 main tiles [128,1024]: 2 fp16 matmuls each + ACT Exp with fused
    row-sum (accum_out -> ACC[128,16]).  spos-tile exps write real fp16
    values; window matmuls (OZ1) accumulate their col sums into rows
    0..3 of a [6,512] psum tile; pos/neg pair logits land in rows 4..5.
  - Outputs: ACC [128,16] and cs [6,512] f32; host finishes in f64.
  - PSUM budget: small pool 1 bank (warm/iv/ssq/cs rotate) + main pool
    3x2 banks = 7 of 8 banks.
"""

import numpy as np

B = 4096
D = 256
P = 2048
NCORES = 8
RG = 4                    # row groups (pos-pair slabs of 512)
CG = 2                    # col groups (neg slabs of 1024)
SLAB = P // RG            # 512 pos pairs per row-group
CGN = P // CG             # 1024 negs per col-group
NCOL = 2 * SLAB + 2 * CGN  # 3072

OFF_SPOS = 0
OFF_SN = 512
OFF_WN = 1536
OFF_WPOS = 2560

ZT_CHUNKS = [(0, 512), (512, 1024), (1536, 1024), (2560, 512)]
NSSQ = NCOL // 512        # 6 ssq rows of 512 cols

_CACHE: dict = {}


def _build_nc():
    import concourse.bacc as bacc
    import concourse.tile as tile
    from concourse import mybir
    from concourse.tile_rust import add_dep_helper

    f32 = mybir.dt.float32
    f16 = mybir.dt.float16
    f8 = mybir.dt.float8e4
    DR = mybir.MatmulPerfMode.DoubleRow
    EXP = mybir.ActivationFunctionType.Exp
    LN = mybir.ActivationFunctionType.Ln

    nc = bacc.Bacc("TRN2", target_bir_lowering=False, debug=False,
                   num_devices=NCORES)
    zt_d = nc.dram_tensor("zt", [128, 2 * NCOL], f16, kind="ExternalInput")
    iv_d = nc.dram_tensor("tauv", [1, 1], f32, kind="ExternalInput")
    obb_d = nc.dram_tensor("obb", [6, 6 * 128], f16, kind="ExternalInput")
    acc_d = nc.dram_tensor("acc", [128, 16], f32, kind="ExternalOutput")
    cs_d = nc.dram_tensor("cs", [6, 512], f32, kind="ExternalOutput")

    zt3 = zt_d[:, :].rearrange("p (h c) -> p h c", h=2)  # [128,2,NCOL] view

    with tile.TileContext(nc) as tc:
        with (
            tc.tile_pool(name="const", bufs=1) as constp,
            tc.tile_pool(name="big", bufs=1) as bigp,
            tc.tile_pool(name="work", bufs=3) as workp,
            tc.tile_pool(name="esc", bufs=8) as escp,
            tc.tile_pool(name="sps", bufs=1, space="PSUM") as sps,
            tc.tile_pool(name="mps", bufs=3, space="PSUM") as mps,
        ):
            # ---------------- input DMAs ----------------
            # zt chunks on the sync queue, depth-2 chained so two are in
            # flight and arrival is staggered for the square/ssq chase.
            # iv + obb ride the gpsimd queue (cheap Pool-side triggers).
            zt = bigp.tile([128, 2, NCOL], f16)
            ivt = constp.tile([1, 1], f32)
            dmas = []
            for s, w in ZT_CHUNKS:
                d = nc.sync.dma_start(zt[:, :, s:s + w], zt3[:, :, s:s + w])
                if len(dmas) >= 2:
                    add_dep_helper(d.ins, dmas[-2].ins, sync=True,
                                   reason="zt chunk ordering")
                dmas.append(d)
            nc.gpsimd.dma_start(ivt[:], iv_d[:])

            # ---------------- constants (gpsimd memsets) ----------------
            ones16_1 = constp.tile([1, 128], f16)
            nc.gpsimd.memset(ones16_1[:], 1.0)
            junkW = constp.tile([128, 128], f16)
            nc.gpsimd.memset(junkW[:], 0.0)
            junkR = constp.tile([128, 512], f16)
            nc.gpsimd.memset(junkR[:], 0.0)
            # EZt: tau at col 6 (windows of width 6 place tau*ssq of
            # 512-chunk r at psum partition r).  OZ1: same but value 1.0
            # for exp col-sums / pair sums.  OBB: one-hot lhsT patterns
            # for the rn partition-broadcast matmuls.
            EZt = constp.tile([128, 12], f16)
            nc.gpsimd.memset(EZt[:], 0.0)
            OZ1 = constp.tile([128, 12], f16)
            nc.gpsimd.memset(OZ1[:], 0.0)
            nc.gpsimd.memset(OZ1[:, 6:7], 1.0)
            # DoubleRow colsum windows, padded to M=32 (DR ldweights
            # rejects tiny M): kt=0 hot at window position m0, kt=1 at
            # m0+1.
            OZD = constp.tile([128, 128], f8)
            nc.gpsimd.memset(OZD[:], 0.0)
            nc.gpsimd.memset(OZD[:, 32:33], 1.0)    # kt=0 hot col 32
            nc.gpsimd.memset(OZD[:, 97:98], 1.0)    # kt=1 hot col 33
            OZD3 = OZD[:, :].rearrange("p (k c) -> p k c", k=2)
            OBB = constp.tile([6, 6 * 128], f16)
            nc.gpsimd.dma_start(OBB[:], obb_d[:])
            dum1 = constp.tile([1, 1], f16)
            nc.gpsimd.memset(dum1[:], 1.0)

            # dummy Ln at t=0: forces the single table load
            # (natural_log_exp_and_others covers Ln AND Exp) during DMA.
            dumo = constp.tile([1, 1], f32)
            nc.scalar.activation(dumo[:], dum1[:], LN)

            # ---------------- PE warm-up + tau bootstrap ----------------
            warm = sps.tile([128, 512], f32, tag="sps")
            for _ in range(8):
                nc.tensor.matmul(warm[:], junkW[:], junkR[:],
                                 start=True, stop=True)
            iv16 = constp.tile([1, 1], f16)
            nc.vector.tensor_copy(iv16[:], ivt[:])
            iv_ps = sps.tile([128, 1], f32, tag="sps")
            nc.tensor.matmul(iv_ps[:], ones16_1[0:1, :], iv16[0:1, 0:1],
                             start=True, stop=True)
            nc.vector.tensor_copy(EZt[:, 6:7], iv_ps[:])

            # ---------------- column sumsq ----------------
            sq = bigp.tile([128, 2, NCOL], f16)
            ssq = sps.tile([6, 512], f32, tag="sps")
            for ci, (s, w) in enumerate(ZT_CHUNKS):
                nc.vector.tensor_mul(sq[:, :, s:s + w], zt[:, :, s:s + w],
                                     zt[:, :, s:s + w])
                lo = s
                while lo < s + w:
                    r = lo // 512
                    for h in range(2):
                        nc.tensor.matmul(
                            ssq[0:6, 0:512], EZt[:, 6 - r:12 - r],
                            sq[:, h, lo:lo + 512],
                            start=(r == 0 and h == 0),
                            stop=(r == NSSQ - 1 and h == 1))
                    lo += 512

            # rn = exp(-0.5 * ln(tau * ssq)) = 1/(n*sqrt(tau)), on ACT only
            lnt = constp.tile([6, 512], f32)
            nc.scalar.activation(lnt[:], ssq[0:6, :], LN)
            rn_t = constp.tile([6, 512], f16)
            nc.scalar.activation(rn_t[:], lnt[:], EXP, scale=-0.5)

            # ---------------- rn partition-broadcast via PE ----------------
            # piece k covers cols [ds,ds+w); rnb stays in PSUM (f32) and
            # normalize reads it directly (DVE 1x from PSUM, no copy).
            # ztn is fp8e4 so the mains can use DoubleRow (K=256 in one
            # matmul); the 1x DVE mode is forced by the PSUM operand
            # anyway, so the fp8 output costs nothing extra.
            ztn = bigp.tile([128, 2, NCOL], f8)
            for (ds, w) in ZT_CHUNKS:
                rp = mps.tile([128, 1024], f32, tag="mps", name=f"rnb{ds}")
                for jj in range(w // 512):
                    j = (ds + 512 * jj) // 512
                    nc.tensor.matmul(
                        rp[:, 512 * jj:512 * jj + 512],
                        OBB[:, 128 * j:128 * j + 128], rn_t[0:6, :],
                        start=True, stop=True)
                for h in range(2):
                    nc.vector.tensor_mul(ztn[:, h, ds:ds + w],
                                         zt[:, h, ds:ds + w],
                                         rp[:, 0:w])

            # ---------------- main tiles ----------------
            # spos tiles first (their exps feed colsums); all colsum
            # matmuls are DEFERRED into the wpos phase so the PE is never
            # a per-tile bottleneck while its clock is still ramping.
            ACC = constp.tile([128, 16], f32)
            escJ = constp.tile([128, 1024], f8)    # wpos exp sink
            cs = sps.tile([32, 512], f32, tag="sps")
            cs_started = [False]
            escs = []

            def cs_mm(row, rhs_ap, stop=False):
                nc.tensor.matmul(cs[0:6, 0:rhs_ap.shape[-1]],
                                 OZ1[:, 6 - row:12 - row], rhs_ap,
                                 start=not cs_started[0], stop=stop,
                                 skip_group_check=True)
                cs_started[0] = True

            def cs_mm_dr(u, esc, stop=False):
                # one DoubleRow matmul col-sums BOTH 512-chunks of an esc
                # tile: kt=0 hot at row 2u, kt=1 hot at row 2u+1 (rows
                # 6..31 accumulate zeros into the padded cs tile).
                e3 = esc[:, :].rearrange("p (k n) -> p k n", k=2)
                m0 = 2 * u
                nc.tensor.matmul(cs[0:32, 0:512],
                                 OZD3[:, :, 32 - m0:64 - m0], e3[:, :, :],
                                 start=not cs_started[0], stop=stop,
                                 perf_mode=DR, skip_group_check=True)
                cs_started[0] = True

            def main_tile(T, lhs_off, rhs_off, is_spos):
                ps = mps.tile([128, 1024], f32, tag="mps", name=f"mm{T}")
                for c2 in range(2):
                    nc.tensor.matmul(
                        ps[:, 512 * c2:512 * c2 + 512],
                        ztn[:, :, lhs_off:lhs_off + 128],
                        ztn[:, :, rhs_off + 512 * c2:
                            rhs_off + 512 * c2 + 512],
                        start=True, stop=True, perf_mode=DR)
                if is_spos:
                    esc = escp.tile([128, 1024], f8, tag="esc",
                                    name=f"esc{T}")
                    nc.scalar.activation(esc[:], ps[:], EXP,
                                         accum_out=ACC[:, T:T + 1])
                    escs.append(esc)
                else:
                    nc.scalar.activation(escJ[:], ps[:], EXP,
                                         accum_out=ACC[:, T:T + 1])

            for t in range(4):
                for u in range(2):
                    main_tile(2 * t + u, OFF_SPOS + 128 * t,
                              OFF_SN + 1024 * u, True)

            # pair logits (DVE products + window matmuls into cs rows 4/5)
            pr_pos = workp.tile([128, 2, 512], f16, tag="pr")
            nc.vector.tensor_mul(pr_pos[:],
                                 ztn[:, :, OFF_SPOS:OFF_SPOS + 512],
                                 ztn[:, :, OFF_WPOS:OFF_WPOS + 512])
            pr_neg = workp.tile([128, 2, 256], f16, tag="pr")
            nc.vector.tensor_mul(pr_neg[:],
                                 ztn[:, :, OFF_SN:OFF_SN + 256],
                                 ztn[:, :, OFF_WN:OFF_WN + 256])

            for t in range(4):
                for u in range(2):
                    T = 8 + 2 * t + u
                    main_tile(T, OFF_WPOS + 128 * t, OFF_SN + 1024 * u,
                              False)
                    k = T - 8
                    if k < 8:
                        cs_mm_dr(k % 2, escs[k])
            cs_mm(4, pr_pos[:, 0, :])
            cs_mm(4, pr_pos[:, 1, :])
            cs_mm(5, pr_neg[:, 0, :])
            cs_mm(5, pr_neg[:, 1, :], stop=True)

            # ---------------- outputs ----------------
            csb = constp.tile([6, 512], f32)
            nc.vector.tensor_copy(csb[:], cs[0:6, :])
            nc.sync.dma_start(acc_d[:], ACC[:])
            nc.sync.dma_start(cs_d[:], csb[:])

    nc.compile()
    return nc


def get_nc():
    if "nc" not in _CACHE:
        _CACHE["nc"] = _build_nc()
    return _CACHE["nc"]


def make_in_maps(strong: np.ndarray, weak: np.ndarray, temp: np.ndarray):
    """Host-side sharding: slice + roll + transpose (pure data movement)."""
    tauv = np.asarray(temp, np.float32).reshape(1, 1)
    obb = np.zeros((6, 6 * 128), np.float16)
    for j in range(NSSQ):
        obb[j, 128 * j:128 * j + 128] = 1.0
    in_maps = []
    for r in range(RG):
        for g in range(CG):
            spos = strong[SLAB * r:SLAB * r + SLAB]
            wpos = weak[SLAB * r:SLAB * r + SLAB]
            sneg = np.roll(strong[P + CGN * g:P + CGN * g + CGN],
                           -256 * r, axis=0)
            wneg = np.roll(weak[P + CGN * g:P + CGN * g + CGN],
                           -256 * r, axis=0)
            cols = np.concatenate([spos, sneg, wneg, wpos], axis=0)
            zt16 = cols.T.astype(np.float16)              # [256, 3072]
            ztd = np.ascontiguousarray(
                zt16.reshape(2, 128, NCOL).transpose(1, 0, 2)
                .reshape(128, 2 * NCOL))
            in_maps.append({"zt": ztd, "tauv": tauv, "obb": obb})
    return in_maps


def kernel(inputs, strong_inputs, targets, num_pos, temperature):
    assert int(num_pos) == P
    strong = np.ascontiguousarray(np.asarray(strong_inputs, dtype=np.float32))
    weak = np.ascontiguousarray(np.asarray(inputs, dtype=np.float32))
    temp = np.asarray(temperature, dtype=np.float32).reshape(1, 1)

    from concourse.bass_utils import run_bass_kernel_spmd

    nc = get_nc()
    in_maps = make_in_maps(strong, weak, temp)
    res = run_bass_kernel_spmd(nc, in_maps, core_ids=list(range(NCORES)))
    return finish_host(res.results)


def finish_host(results):
    """Final ln(S + e^p) - p reduction in float64 on the host."""
    S1s = np.zeros((RG, SLAB))
    S1w = np.zeros((RG, SLAB))
    CA = np.zeros((CG, CGN))
    CB = np.zeros((CG, CGN))
    pos_l = np.zeros((RG, SLAB))
    neg_l = np.zeros((CG, CGN))
    for r in range(RG):
        for g in range(CG):
            res = results[CG * r + g]
            acc = np.asarray(res["acc"], np.float64)     # [128, 16]
            cs = np.asarray(res["cs"], np.float64)       # [6, 512]
            for t in range(4):
                sl = slice(128 * t, 128 * t + 128)
                S1s[r, sl] += acc[:, 2 * t] + acc[:, 2 * t + 1]
                S1w[r, sl] += acc[:, 8 + 2 * t] + acc[:, 8 + 2 * t + 1]
            CA[g] += np.roll(cs[0:2].reshape(CGN), 256 * r)
            CB[g] += np.roll(cs[2:4].reshape(CGN), 256 * r)
            if g == 0:
                pos_l[r] = cs[4]
            neg_l[g, 256 * r:256 * r + 256] = cs[5, 0:256]
    p = pos_l.reshape(-1)
    q = neg_l.reshape(-1)
    ep, eq = np.exp(p), np.exp(q)
    total = (np.sum(np.log(S1s.reshape(-1) + ep) - p)
             + np.sum(np.log(S1w.reshape(-1) + ep) - p)
             + np.sum(np.log(CA.reshape(-1) + eq) - q)
             + np.sum(np.log(CB.reshape(-1) + eq) - q))
    return np.float32(total / (2 * B))
